# revision 35
# baseline (speedup 1.0000x reference)
"""Bahdanau attention + LayerNorm + residual, Trainium2 Bass kernel.

Problem shapes (hardcoded): B=8, Tx=Ty=128, D=H=512, fp32 I/O.

Sharding: data-parallel over batch B across the 8 NeuronCores (one batch
element per core, no collectives).  Weights are replicated to every core.

Per-core algorithm:
  WcT[h,x]  = sum_d Wa[d,h]*ctx[x,d] + (bWa+bUa)[h]     (PE, 4 h-chunks of 128)
  UxT[h,y]  = sum_d Ua[d,h]*x[y,d]                      (PE)
  targ[h,(y,x)] = WcT[h,x] + UxT[h,y]                   (DVE broadcast add)
  t = tanh(targ)                                        (ACT, giant in-place)
  scores[y,x] = sum_h Va[h]*t[h,(y,x)]                  (PE -> PSUM quadrants)
  attn = softmax over x (exp shares the ACT table set with tanh)
  cv = attn @ ctx                                       (PE)
  out = LN(cv)*gamma + beta + x   (bn_stats/bn_aggr + Newton rsqrt, all DVE)

bVa provably cancels in softmax (constant shift) and is unused.

The T pipeline runs in bfloat16: the broadcast-add uses a pair-duplicated
UxT2 layout so both DVE read streams have innermost step-1 16-bit pairs ->
2x_1P mode (~245 Gelem/s); tanh on ACT is dtype-independent (1 elem/lane/cyc
@1.2GHz) so ACT is the bottleneck, which is the hardware floor (~55us/core).
"""

import numpy as np

B, TX, TY, D, H = 8, 128, 128, 512, 512
LN_EPS = 1e-3
NCORES = 8
P = 128           # partitions
HCHUNKS = H // P  # 4
DCHUNKS = D // P  # 4
NYH = TY // 2     # 64 y rows per half

COMPUTE_DT = "bfloat16"   # T-pipeline dtype: "bfloat16" or "float32"
DEBUG_DUMPS = False      # add dbg_sc output (raw scores) for HW bisection

RSQRT_MAGIC = 0x5F3759DF


def _build_nc(compute_dt_name: str):
    import concourse.bass as bass
    import concourse.bacc as bacc
    import concourse.mybir as mybir
    from concourse.tile import TileContext
    from contextlib import ExitStack

    f32 = mybir.dt.float32
    i32 = mybir.dt.int32
    cdt = getattr(mybir.dt, compute_dt_name)
    AF = mybir.ActivationFunctionType
    OP = mybir.AluOpType
    X = mybir.AxisListType.X

    nc = bacc.Bacc()

    ctx_d = nc.dram_tensor("context", [TX, D], f32, kind="ExternalInput")
    x_d = nc.dram_tensor("x", [TY, D], f32, kind="ExternalInput")
    wa_d = nc.dram_tensor("Wa", [D, H], f32, kind="ExternalInput")
    ua_d = nc.dram_tensor("Ua", [D, H], f32, kind="ExternalInput")
    va_d = nc.dram_tensor("Va", [H, 1], f32, kind="ExternalInput")
    bwa_d = nc.dram_tensor("bWa", [H], f32, kind="ExternalInput")
    bua_d = nc.dram_tensor("bUa", [H], f32, kind="ExternalInput")
    gamma_d = nc.dram_tensor("gamma", [D], f32, kind="ExternalInput")
    beta_d = nc.dram_tensor("beta", [D], f32, kind="ExternalInput")
    ident_d = nc.dram_tensor("ident", [P, P], f32, kind="ExternalInput")
    out_d = nc.dram_tensor("out", [TY, D], f32, kind="ExternalOutput")
    dbg_sc_d = None
    if DEBUG_DUMPS:
        dbg_sc_d = nc.dram_tensor("dbg_sc", [TY, TX], f32, kind="ExternalOutput")
    # DRAM bounce buffers for the score gather: linear order (q, t, j, x)
    # == scores row-major, so both DMAs have trivial access patterns.
    sgather_d = [
        nc.dram_tensor(f"sgather{h}", [NYH, TX], f32) for h in range(2)
    ]

    wa_r = None  # set below

    with TileContext(nc) as tc, ExitStack() as ctx:
        persist = ctx.enter_context(tc.tile_pool(name="persist", bufs=1))
        wpool = ctx.enter_context(tc.tile_pool(name="wpool", bufs=1))
        targ_pool = ctx.enter_context(tc.tile_pool(name="targ", bufs=8))
        epi_pool = ctx.enter_context(tc.tile_pool(name="epi", bufs=2))
        sp_pool = ctx.enter_context(tc.tile_pool(name="spsum", bufs=1, space="PSUM"))
        pp_pool = ctx.enter_context(tc.tile_pool(name="pp", bufs=3, space="PSUM"))

        # ---------------- prologue: loads (critical path first) -------------
        # ctx/x as 4 column tiles each (parallel DMA queues, gate transposes)
        ctx_col = []
        x_col = []
        for dc in range(DCHUNKS):
            t = persist.tile([P, P], f32, name=f"ctxcol{dc}")
            nc.sync.dma_start(out=t[:], in_=ctx_d[:, dc * P:(dc + 1) * P])
            ctx_col.append(t)
        ident_sb = persist.tile([P, P], f32)
        nc.sync.dma_start(out=ident_sb[:], in_=ident_d[:])

        # first weight chunk split by d for the fastest possible start
        wa0 = []
        ua0 = []
        for dc in range(DCHUNKS):
            t = wpool.tile([P, P], f32, name=f"wa0_{dc}")
            nc.sync.dma_start(
                out=t[:], in_=wa_d[dc * P:(dc + 1) * P, 0:P]
            )
            wa0.append(t)
        for dc in range(DCHUNKS):
            t = x_col_t = persist.tile([P, P], f32, name=f"xcol{dc}")
            nc.sync.dma_start(out=t[:], in_=x_d[:, dc * P:(dc + 1) * P])
            x_col.append(t)
        for dc in range(DCHUNKS):
            t = wpool.tile([P, P], f32, name=f"ua0_{dc}")
            nc.sync.dma_start(
                out=t[:], in_=ua_d[dc * P:(dc + 1) * P, 0:P]
            )
            ua0.append(t)

        # bias sum (bWa + bUa) as [128, 4]
        bwa_sb = persist.tile([P, HCHUNKS], f32)
        nc.sync.dma_start(out=bwa_sb[:], in_=bwa_d[:].rearrange("(c p) -> p c", p=P))
        bua_sb = persist.tile([P, HCHUNKS], f32)
        nc.sync.dma_start(out=bua_sb[:], in_=bua_d[:].rearrange("(c p) -> p c", p=P))
        bsum_sb = persist.tile([P, HCHUNKS], f32)
        nc.vector.tensor_tensor(bsum_sb[:], bwa_sb[:], bua_sb[:], OP.add)

        # remaining weight chunks (one DMA per h-chunk)
        wa_r = wa_d[:].rearrange("(dc dp) h -> dp dc h", dp=P)
        ua_r = ua_d[:].rearrange("(dc dp) h -> dp dc h", dp=P)
        wa_h = {}
        ua_h = {}
        for hc in range(1, HCHUNKS):
            t = wpool.tile([P, DCHUNKS, P], f32, name=f"wah{hc}")
            nc.sync.dma_start(out=t[:], in_=wa_r[:, :, hc * P:(hc + 1) * P])
            wa_h[hc] = t
            t = wpool.tile([P, DCHUNKS, P], f32, name=f"uah{hc}")
            nc.sync.dma_start(out=t[:], in_=ua_r[:, :, hc * P:(hc + 1) * P])
            ua_h[hc] = t

        # Va as [128, 4]: va32[p, c] = Va[c*128+p]
        va32 = persist.tile([P, HCHUNKS], f32)
        nc.sync.dma_start(out=va32[:], in_=va_d[:, 0].rearrange("(c p) -> p c", p=P))
        va_sb = persist.tile([P, HCHUNKS], cdt)
        nc.vector.tensor_copy(va_sb[:], va32[:])

        # gamma/beta broadcast over 64 partitions + per-half (beta + x)
        gamma64 = persist.tile([NYH, D], f32)
        nc.sync.dma_start(out=gamma64[:], in_=gamma_d[:].partition_broadcast(NYH))
        beta64 = persist.tile([NYH, D], f32)
        nc.sync.dma_start(out=beta64[:], in_=beta_d[:].partition_broadcast(NYH))
        xh = []
        bxh = []
        for h in range(2):
            t = persist.tile([NYH, D], f32, name=f"xh{h}")
            nc.sync.dma_start(out=t[:], in_=x_d[h * NYH:(h + 1) * NYH, :])
            xh.append(t)
            bt = persist.tile([NYH, D], f32, name=f"bxh{h}")
            nc.vector.tensor_tensor(bt[:], beta64[:], t[:], OP.add)
            bxh.append(bt)

        # transposes of ctx and x: ctxT[dc][d',x] = ctx[x, dc*128+d']
        ctxT = []
        xT = []
        for dc in range(DCHUNKS):
            pt = pp_pool.tile([P, P], f32, tag="pp", name=f"tp_ctx{dc}")
            nc.tensor.transpose(pt[:], ctx_col[dc][:], ident_sb[:])
            t = persist.tile([P, P], f32, name=f"ctxT{dc}")
            nc.vector.tensor_copy(t[:], pt[:])
            ctxT.append(t)
        for dc in range(DCHUNKS):
            pt = pp_pool.tile([P, P], f32, tag="pp", name=f"tp_x{dc}")
            nc.tensor.transpose(pt[:], x_col[dc][:], ident_sb[:])
            t = persist.tile([P, P], f32, name=f"xT{dc}")
            nc.vector.tensor_copy(t[:], pt[:])
            xT.append(t)

        # WcT / UxT2 per h-chunk (PE matmuls -> DVE copies to bf16)
        wct = []
        uxt2 = []
        for hc in range(HCHUNKS):
            pw = pp_pool.tile([P, P], f32, tag="pp", name=f"pw{hc}")
            for dc in range(DCHUNKS):
                lhs = wa0[dc][:] if hc == 0 else wa_h[hc][:, dc, :]
                nc.tensor.matmul(
                    pw[:], lhs, ctxT[dc][:],
                    start=(dc == 0), stop=(dc == DCHUNKS - 1),
                )
            w_t = persist.tile([P, P], cdt, name=f"wct{hc}")
            nc.vector.tensor_scalar(
                w_t[:], pw[:], bsum_sb[:, hc:hc + 1], None, OP.add
            )
            wct.append(w_t)

            pu = pp_pool.tile([P, P], f32, tag="pp", name=f"pu{hc}")
            for dc in range(DCHUNKS):
                lhs = ua0[dc][:] if hc == 0 else ua_h[hc][:, dc, :]
                nc.tensor.matmul(
                    pu[:], lhs, xT[dc][:],
                    start=(dc == 0), stop=(dc == DCHUNKS - 1),
                )
            u_t = persist.tile([P, 2 * P], cdt, name=f"uxt{hc}")
            # duplicate each y value twice: u_t[p, 2y+i] = UxT[p, y]
            nc.vector.tensor_copy(
                u_t[:].rearrange("p (y two) -> p y two", two=2),
                pu[:].unsqueeze(2).broadcast_to([P, P, 2]),
            )
            uxt2.append(u_t)

        # score accumulators for the current y-half: 4 one-bank tiles, each
        # holding 4 groups at partition quadrants {0, 32, 64, 96} (replicated
        # to 32 rows).  group g (0..15) covers y rows 4g..4g+3 of the half:
        # tile = g % 4, quadrant = g // 4, i.e. y = 16q + 4t + j.
        spsum = [None] * 4

        UNIT_Y = 32  # y rows per unit; all 4 h-chunk tiles of a unit coexist

        def emit_unit(half, u):
            """For y rows [32u, 32u+32) of the half: the 4 h-chunks' tanh-arg
            adds + tanhs, then the score matmuls with each group's 4-chunk
            PSUM accumulation CONSECUTIVE (hardware `start` clears the
            has_written state of the whole 2KB zero region, so two
            accumulation groups must never interleave within one bank)."""
            y0 = u * UNIT_Y
            tiles = []
            for c in range(HCHUNKS):
                targ = targ_pool.tile(
                    [P, UNIT_Y * TX], cdt, tag="targ", name=f"targ{c}"
                )
                in0 = (
                    wct[c][:]
                    .rearrange("p (xh two) -> p xh two", two=2)
                    .unsqueeze(1)
                    .broadcast_to([P, UNIT_Y, P // 2, 2])
                )
                off = (half * NYH + y0) * 2
                in1 = (
                    uxt2[c][:, off:off + 2 * UNIT_Y]
                    .rearrange("p (y two) -> p y two", two=2)
                    .unsqueeze(2)
                    .broadcast_to([P, UNIT_Y, P // 2, 2])
                )
                out_ap = targ[:].rearrange(
                    "p (y xh two) -> p y xh two", y=UNIT_Y, two=2
                )
                nc.vector.tensor_tensor(out_ap, in0, in1, OP.add)
                nc.scalar.activation(targ[:], targ[:], AF.Tanh)
                tiles.append(targ)
            # unit u covers quadrants {2u, 2u+1}: y = 16q + 4t + j
            for t in range(4):
                for q in (2 * u, 2 * u + 1):
                    row = 32 * q
                    for c in range(HCHUNKS):
                        yloc = 16 * q + 4 * t - y0  # row block in this unit
                        nc.tensor.matmul(
                            spsum[t][row:row + 32, :],
                            va_sb[:, c:c + 1].broadcast_to([P, 32]),
                            tiles[c][:, yloc * TX:(yloc + 4) * TX],
                            start=(c == 0), stop=(c == HCHUNKS - 1),
                            tile_position=(0, row),
                        )

        # copy phase: drain score PSUM tiles to SBUF right after each half's
        # last matmuls, releasing the PSUM banks for the next half quickly.
        sp_sb_h = [None, None]
        out_tiles = [None, None]

        def epilogue_copy(h):
            use_act = (h == 1)  # ACT is idle at the very end; busy otherwise
            sp_sb = epi_pool.tile([P, 4, 512], f32, tag="spsb", name="spsb")
            for t in range(4):
                if use_act and t % 2 == 0:
                    nc.scalar.copy(sp_sb[:, t, :], spsum[t][:])
                else:
                    nc.vector.tensor_copy(sp_sb[:, t, :], spsum[t][:])
            sp_sb_h[h] = sp_sb

        def epilogue_rest(h):
            sp_sb = sp_sb_h[h]
            sc = epi_pool.tile([NYH, TX], f32, tag="scores", name="sc")
            # rows {0,32,64,96} of sp_sb -> DRAM (contiguous per row); the
            # DRAM linear order (q, t, j, x) equals scores[y, x] row-major.
            src = sp_sb[:].rearrange("(q r) t w -> q r t w", r=32)[:, 0, :, :]
            nc.sync.dma_start(
                out=sgather_d[h][:].rearrange("(q tj) x -> q (tj x)", q=4),
                in_=src.rearrange("q t w -> q (t w)"),
            )
            nc.sync.dma_start(out=sc[:], in_=sgather_d[h][:])
            if DEBUG_DUMPS:
                nc.sync.dma_start(
                    out=dbg_sc_d[h * NYH:(h + 1) * NYH, :], in_=sc[:]
                )

            nmax = epi_pool.tile([NYH, 1], f32, tag="nmax", name="nmax")
            nc.vector.tensor_reduce(nmax[:], sc[:], axis=X, op=OP.max, negate=True)
            e_t = epi_pool.tile([NYH, TX], f32, tag="et", name="et")
            nc.scalar.activation(e_t[:], sc[:], AF.Exp, bias=nmax[:, 0:1], scale=1.0)
            sume = epi_pool.tile([NYH, 1], f32, tag="sume", name="sume")
            nc.vector.tensor_reduce(sume[:], e_t[:], axis=X, op=OP.add)
            rsum = epi_pool.tile([NYH, 1], f32, tag="rsum", name="rsum")
            nc.vector.reciprocal(rsum[:], sume[:])
            nc.vector.tensor_scalar(e_t[:], e_t[:], rsum[:, 0:1], None, OP.mult)

            # cv = attn @ ctx   (transpose attn, then 4 column matmuls
            # against the ctx column tiles)
            etp = pp_pool.tile([TX, NYH], f32, tag="pp", name="etp")
            nc.tensor.transpose(etp[:], e_t[:], ident_sb[:NYH, :NYH])
            et_sb = epi_pool.tile([TX, NYH], f32, tag="etsb", name="etsb")
            nc.vector.tensor_copy(et_sb[:], etp[:])
            cv_ps = pp_pool.tile([NYH, D], f32, tag="pp", name="cvps")
            for dc in range(DCHUNKS):
                nc.tensor.matmul(
                    cv_ps[:, dc * P:(dc + 1) * P], et_sb[:], ctx_col[dc][:],
                    start=True, stop=True,
                )

            # LayerNorm: bn stats + Newton rsqrt (all DVE; no ACT table
            # switch, so tanh/exp's table set stays resident)
            stats = epi_pool.tile([NYH, 6], f32, tag="bns", name="bns")
            nc.vector.bn_stats(out=stats[:], in_=cv_ps[:])
            mv = epi_pool.tile([NYH, 2], f32, tag="mv", name="mv")
            nc.vector.bn_aggr(out=mv[:], in_=stats[:])
            v_t = epi_pool.tile([NYH, 1], f32, tag="veps", name="veps")
            nc.vector.tensor_scalar(v_t[:], mv[:, 1:2], LN_EPS, None, OP.add)
            # magic-constant seed: y0 = bits(0x5f3759df - (bits(v) >> 1))
            ib = epi_pool.tile([NYH, 1], i32, tag="ib", name="ib")
            nc.vector.tensor_scalar(
                ib[:], v_t[:].bitcast(i32), 1, None, OP.logical_shift_right
            )
            nc.vector.tensor_scalar(ib[:], ib[:], -1, RSQRT_MAGIC, OP.mult, OP.add)
            y_t = epi_pool.tile([NYH, 1], f32, tag="yrs", name="yrs")
            nc.vector.tensor_copy(y_t[:], ib[:].bitcast(f32))
            tmp = epi_pool.tile([NYH, 1], f32, tag="tnw", name="tnw")
            for _ in range(3):  # Newton: y *= 1.5 - 0.5*v*y^2
                nc.vector.tensor_tensor(tmp[:], y_t[:], y_t[:], OP.mult)
                nc.vector.tensor_tensor(tmp[:], tmp[:], v_t[:], OP.mult)
                nc.vector.tensor_scalar(tmp[:], tmp[:], -0.5, 1.5, OP.mult, OP.add)
                nc.vector.tensor_tensor(y_t[:], y_t[:], tmp[:], OP.mult)

            cvn = epi_pool.tile([NYH, D], f32, tag="cvn", name="cvn")
            nc.vector.tensor_scalar(
                cvn[:], cv_ps[:], mv[:, 0:1], y_t[:, 0:1], OP.subtract, OP.mult
            )
            o_t = epi_pool.tile([NYH, D], f32, tag="otile", name="otile")
            nc.vector.tensor_tensor(o_t[:], cvn[:], gamma64[:], OP.mult)
            nc.vector.tensor_tensor(o_t[:], o_t[:], bxh[h][:], OP.add)
            out_tiles[h] = o_t

        # ---------------- main loop (half-outer) ----------------
        for half in range(2):
            for t in range(4):
                spsum[t] = sp_pool.tile(
                    [P, 512], f32, tag=f"sp{t}", name=f"spsum{t}_{half}"
                )
            emit_unit(half, 0)
            # emit half-0's epilogue tail mid-half-1 so the DVE priority
            # order keeps ACT fed (deps allow full overlap)
            if half == 1:
                epilogue_rest(0)
            emit_unit(half, 1)
            epilogue_copy(half)
        epilogue_rest(1)
        for h in range(2):
            nc.sync.dma_start(
                out=out_d[h * NYH:(h + 1) * NYH, :], in_=out_tiles[h][:]
            )

    nc.compile()  # bacc passes: wait splitting (HW allows 1 wait/instr), etc.
    return nc


_NC_CACHE = {}


def _get_nc():
    key = COMPUTE_DT
    if key not in _NC_CACHE:
        _NC_CACHE[key] = _build_nc(key)
    return _NC_CACHE[key]


def _in_maps(inputs):
    inputs = {k: np.asarray(v, dtype=np.float32) for k, v in inputs.items()}
    ident = np.eye(P, dtype=np.float32)
    maps = []
    for b in range(NCORES):
        maps.append({
            "context": np.ascontiguousarray(inputs["context"][b]),
            "x": np.ascontiguousarray(inputs["x"][b]),
            "Wa": inputs["Wa"],
            "Ua": inputs["Ua"],
            "Va": inputs["Va"].reshape(H, 1),
            "bWa": inputs["bWa"],
            "bUa": inputs["bUa"],
            "gamma": inputs["gamma"],
            "beta": inputs["beta"],
            "ident": ident,
        })
    return maps


def kernel(**inputs) -> np.ndarray:
    from concourse.bass_utils import run_bass_kernel_spmd

    nc = _get_nc()
    res = run_bass_kernel_spmd(nc, _in_maps(inputs), core_ids=list(range(NCORES)))
    return np.stack([res.results[i]["out"] for i in range(NCORES)], axis=0)


def run_timed(inputs, trace=False, **kw):
    """Returns (output [B,TY,D], BassKernelResults)."""
    from concourse.bass_utils import run_bass_kernel_spmd

    nc = _get_nc()
    res = run_bass_kernel_spmd(
        nc, _in_maps(inputs), core_ids=list(range(NCORES)), trace=trace, **kw
    )
    out = np.stack([res.results[i]["out"] for i in range(NCORES)], axis=0)
    return out, res
